# revision 1
# baseline (speedup 1.0000x reference)
"""nn_AdditiveAttention_755914244534 — Trainium2 Bass kernel.

Math: the reference computes softmax over a trailing size-1 axis, which is
identically 1.0, so out[b, n, :] == values[b, 0, :] for every n. The kernel
is a pure broadcast of `values` (B, 1, DV) to (B, N, DV); queries/keys and
the weights do not affect the output.

Strategy: shard batch B=32 across 8 cores (4 per core). Each core
broadcast-loads its 4 value rows into SBUF (all 128 partitions), then issues
4 large store DMAs (8 MiB each) with step-0 source APs to materialize its
(4, 4096, 512) output shard. HBM-write-bound: ~32 MiB/core.
"""

import numpy as np

from concourse import bass, mybir
from concourse.bass_utils import run_bass_kernel_spmd

B, N, DV = 32, 4096, 512
NCORES = 8
BPC = B // NCORES  # 4 batches per core
P = 128
R = N // P  # 32 repeats of each value row per partition


def build_bass():
    nc = bass.Bass()
    vals = nc.declare_dram_parameter(
        "values", [BPC, DV], mybir.dt.float32, isOutput=False
    )
    out = nc.declare_dram_parameter(
        "out", [BPC, N, DV], mybir.dt.float32, isOutput=True
    )
    with (
        nc.sbuf_tensor([P, BPC * DV], mybir.dt.float32) as t,
        nc.semaphore("dma_sem") as sem,
        nc.Block() as block,
    ):

        @block.sync
        def _(sync):
            # Load all 4 value rows, replicated into all 128 partitions.
            sync.dma_start(
                t[:].rearrange("p (b d) -> p b d", d=DV),
                vals[:].unsqueeze(0).to_broadcast((P, BPC, DV)),
            ).then_inc(sem, 16)
            sync.wait_ge(sem, 16)
            # Store: per batch, one 8 MiB DMA; partition p covers output
            # rows [p*R, (p+1)*R), each row a copy of the value row.
            for b in range(BPC):
                sync.dma_start(
                    out[b].rearrange("(p r) d -> p r d", r=R),
                    t[:, b * DV : (b + 1) * DV]
                    .unsqueeze(1)
                    .to_broadcast((P, R, DV)),
                ).then_inc(sem, 16)
            sync.wait_ge(sem, 16 + 16 * BPC)
    return nc


def run(values: np.ndarray, trace: bool = False):
    """values: full (B, 1, DV) float32. Returns (results, exec_time_ns)."""
    nc = build_bass()
    v = np.ascontiguousarray(values, dtype=np.float32).reshape(B, DV)
    in_maps = [
        {"values": v[c * BPC : (c + 1) * BPC]} for c in range(NCORES)
    ]
    res = run_bass_kernel_spmd(
        nc, in_maps, core_ids=list(range(NCORES)), trace=trace
    )
    return res


def kernel(**inputs: np.ndarray) -> np.ndarray:
    res = run(inputs["values"], trace=False)
    return np.concatenate([r["out"] for r in res.results], axis=0)


# revision 2
# speedup vs baseline: 1.0473x; 1.0473x over previous
"""nn_AdditiveAttention_755914244534 — Trainium2 Bass kernel (8 cores).

Math: the reference's softmax runs over a trailing size-1 axis, so the
attention weights are exactly 1.0 and out[b, n, :] == values[b, 0, :] for
every n — independent of queries/keys/W_q/W_k/w_v. The kernel is a pure
broadcast of `values` (B, 1, DV) to (B, N, DV), bit-exact vs the reference.

Distribution: batch 32 is sharded 4-per-core across the 8 NeuronCores (pure
data parallel, no collectives). Each core materializes its (4, 4096, 512)
f32 shard = 32 MiB of HBM writes; at the ~27 GB/s/engine SBUF-port line rate
of the 16 SDMA engines (~436 GB/s), that is ~79 us of streaming — the
roofline for this problem.

Per-core schedule (all DMAs on the sync-engine HWDGE ring):
  1. load b0's value row broadcast into all 128 partitions (256 KiB),
  2. load rows b1-b3 likewise (768 KiB) — fills the ring while (1) completes,
  3. "direct" store: batch 0's first 8 rows/partition straight from the
     loaded rows (2 KiB descriptors) — starts ~3 us into the block, no
     dependence on the Vector engine,
  4. the Vector engine meanwhile replicates each value row 8x within each
     partition (tb tiles, 16 KiB contiguous chunks),
  5. remaining 31.5 MiB streams from tb with 16 KiB descriptors (line rate).
Semaphores: separate sems for the two loads (DMA completion order is not
FIFO), vsem counts DVE replication, dma_sem counts stores.
"""

import numpy as np

from concourse import bass, mybir
from concourse.bass_utils import run_bass_kernel_spmd

B, N, DV = 32, 4096, 512
NCORES = 8
BPC = B // NCORES  # 4 batches per core
P = 128
R = N // P  # 32 value-row copies per partition
K = 8  # replication factor inside SBUF (store descriptor = K*2 KiB)
R_DIRECT = 8  # rows per partition covered by the fast direct store (2 MiB)


def build_bass():
    nc = bass.Bass()
    vals = nc.declare_dram_parameter(
        "values", [BPC, DV], mybir.dt.float32, isOutput=False
    )
    out = nc.declare_dram_parameter(
        "out", [BPC, N, DV], mybir.dt.float32, isOutput=True
    )
    with (
        nc.sbuf_tensor([P, BPC * DV], mybir.dt.float32) as ts,
        nc.sbuf_tensor([P, BPC * K * DV], mybir.dt.float32) as tb,
        nc.semaphore("dma_sem") as sem,
        nc.semaphore("l0sem") as l0sem,
        nc.semaphore("lrsem") as lrsem,
        nc.semaphore("vsem") as vsem,
        nc.Block(no_gpsimd_drain=True) as block,
    ):

        @block.sync
        def _(sync):
            sync.dma_start(
                ts[:, :DV].unsqueeze(1),
                vals[:1].unsqueeze(0).to_broadcast((P, 1, DV)),
            ).then_inc(l0sem, 16)
            sync.dma_start(
                ts[:, DV:].rearrange("p (b d) -> p b d", d=DV),
                vals[1:].unsqueeze(0).to_broadcast((P, BPC - 1, DV)),
            ).then_inc(lrsem, 16)
            sync.wait_ge(l0sem, 16)
            sync.dma_start(
                out[0].rearrange("(p r) d -> p r d", r=R)[:, :R_DIRECT],
                ts[:, :DV].unsqueeze(1).to_broadcast((P, R_DIRECT, DV)),
            ).then_inc(sem, 16)
            sync.wait_ge(vsem, 1)
            sync.dma_start(
                out[0]
                .rearrange("(p r) d -> p r d", r=R)[:, R_DIRECT:]
                .rearrange("p (q e) d -> p q (e d)", e=K),
                tb[:, : K * DV]
                .unsqueeze(1)
                .to_broadcast((P, (R - R_DIRECT) // K, K * DV)),
            ).then_inc(sem, 16)
            for b in range(1, BPC):
                sync.wait_ge(vsem, b + 1)
                sync.dma_start(
                    out[b]
                    .rearrange("(p r) d -> p r d", r=R)
                    .rearrange("p (q e) d -> p q (e d)", e=K),
                    tb[:, b * K * DV : (b + 1) * K * DV]
                    .unsqueeze(1)
                    .to_broadcast((P, R // K, K * DV)),
                ).then_inc(sem, 16)
            sync.wait_ge(sem, 16 * (BPC + 1))
            sync.wait_ge(lrsem, 16)

        @block.vector
        def _(vector):
            vector.wait_ge(l0sem, 16)
            vector.tensor_copy(
                tb[:, : K * DV].rearrange("p (r d) -> p r d", d=DV),
                ts[:, :DV].unsqueeze(1).to_broadcast((P, K, DV)),
            ).then_inc(vsem, 1)
            vector.wait_ge(lrsem, 16)
            for b in range(1, BPC):
                vector.tensor_copy(
                    tb[:, b * K * DV : (b + 1) * K * DV].rearrange(
                        "p (r d) -> p r d", d=DV
                    ),
                    ts[:, b * DV : (b + 1) * DV].unsqueeze(1).to_broadcast((P, K, DV)),
                ).then_inc(vsem, 1)
    return nc


def run(values: np.ndarray, trace: bool = False):
    """values: full (B, 1, DV) float32. Returns BassKernelResults."""
    nc = build_bass()
    v = np.ascontiguousarray(values, dtype=np.float32).reshape(B, DV)
    in_maps = [{"values": v[c * BPC : (c + 1) * BPC]} for c in range(NCORES)]
    return run_bass_kernel_spmd(
        nc, in_maps, core_ids=list(range(NCORES)), trace=trace
    )


def kernel(**inputs: np.ndarray) -> np.ndarray:
    res = run(inputs["values"], trace=False)
    return np.concatenate([r["out"] for r in res.results], axis=0)
